# revision 1
# baseline (speedup 1.0000x reference)
"""Multi-head attention (b=2, t=2048, d=1024, h=16, hd=64) on 8 trn2 NeuronCores.

Sharding: core c = 4*b + g handles batch b and head-group g (4 heads,
feature columns [g*256, (g+1)*256)). QKV weights column-sharded, Wo
row-sharded (Megatron); each core returns two partial [2048, 1024]
outputs (head-pair 0 / head-pair 1 of its group) that the host sums,
plus bo.

Datapath: fp16 operands (x, Wq/Wk/Wv, Q^T, K^T, V, probs) with fp32
PSUM accumulation everywhere; the context normalize and output
projection run in f32r (TF32-class). Softmax skips max-subtraction:
scores are q.k/8 with q,k ~ N(0,1), far inside exp's range.

Attention is ACT(exp)-bound, so PE work from other phases (Q/K fb1
projections, V transposes via DMA-xbar, pair-0 output projection) is
interleaved into the attention loops to keep the PE HAM clock-gate
warm (cold K=4/8 halves the PE clock).
"""

import numpy as np

import concourse.bass as bass
import concourse.mybir as mybir
import concourse.tile as tile
from concourse.bass_utils import run_bass_kernel_spmd
from concourse.masks import make_identity

F32 = mybir.dt.float32
F32R = mybir.dt.float32r
F16 = mybir.dt.float16
EXP = mybir.ActivationFunctionType.Exp

T = 2048          # tokens per batch
D = 1024          # model dim
HG = 4            # heads per core
HD = 64           # head dim
GF = HG * HD      # 256 features per head-group
VW = HG * (HD + 1)  # 260: V columns + a ones column per head
NT = T // 128     # 16 token blocks

MAX_WAITS = 1


def _split_waits(nc):
    """walrus in this container allows only one sync-wait per instruction;
    hoist extras onto same-engine NoOps immediately before the offender."""
    for f in nc.m.functions:
        for blk in f.blocks:
            insts = list(blk.instructions)
            new, changed = [], False
            for ins in insts:
                si = ins.sync_info
                waits = list(si.on_wait) if si and si.on_wait else []
                if len(waits) > MAX_WAITS:
                    changed = True
                    extra, keep = waits[:-MAX_WAITS], waits[-MAX_WAITS:]
                    for i in range(0, len(extra), MAX_WAITS):
                        new.append(mybir.InstNoOp(
                            name=f"{ins.name}-wsplit{i}",
                            engine=ins.engine,
                            sync_info=mybir.SyncInfo(
                                on_wait=extra[i:i + MAX_WAITS], on_update=[]),
                        ))
                    ins.sync_info = mybir.SyncInfo(
                        on_wait=keep,
                        on_update=list(si.on_update) if si.on_update else [])
                new.append(ins)
            if changed:
                blk.instructions = new


def _build_program():
    nc = bass.Bass("TRN2", target_bir_lowering=False, debug=False, num_devices=8)

    xT = nc.dram_tensor("xT", [D, T], F16, kind="ExternalInput")
    Wq = nc.dram_tensor("Wq", [D, GF], F16, kind="ExternalInput")
    Wk = nc.dram_tensor("Wk", [D, GF], F16, kind="ExternalInput")
    Wv = nc.dram_tensor("Wv", [D, GF], F16, kind="ExternalInput")
    Wo = nc.dram_tensor("Wo", [GF, D], F32R, kind="ExternalInput")
    bq = nc.dram_tensor("bq", [GF, 1], F32, kind="ExternalInput")
    bk = nc.dram_tensor("bk", [GF, 1], F32, kind="ExternalInput")
    bv = nc.dram_tensor("bv", [GF, 1], F32, kind="ExternalInput")
    # single output holding both head-pair partials: [pair*T + t, D]
    out = nc.dram_tensor("out", [2 * T, D], F32, kind="ExternalOutput")

    with tile.TileContext(nc) as tc:
        with (
            nc.allow_low_precision(reason="fp16/f32r rounding is intentional"),
            tc.tile_pool(name="w", bufs=1) as wp,       # persistent tiles
            tc.tile_pool(name="xt", bufs=8) as xp,      # xT tiles
            tc.tile_pool(name="pt", bufs=8) as ptp,     # probs tiles
            tc.tile_pool(name="ob", bufs=3) as obp,     # out staging
            tc.tile_pool(name="ps", bufs=2, space="PSUM") as ps,    # "sp" slots
            tc.tile_pool(name="pst", bufs=2, space="PSUM") as pst,  # S tiles
            tc.tile_pool(name="psc", bufs=1, space="PSUM") as psc,  # C accum
        ):
            # ---- input DMAs ------------------------------------------------
            xT_t, Wq_t, Wk_t, Wv_t = [], [], [], []
            for dc in range(8):
                xt = xp.tile([128, T], F16, tag="xt")
                nc.sync.dma_start(xt[:], xT[dc * 128:(dc + 1) * 128, :])
                xT_t.append(xt)
                for (lst, src, nm) in ((Wq_t, Wq, "wq"), (Wk_t, Wk, "wk"),
                                       (Wv_t, Wv, "wv")):
                    w = wp.tile([128, GF], F16, tag=f"{nm}{dc}", name=f"{nm}{dc}")
                    nc.sync.dma_start(w[:], src[dc * 128:(dc + 1) * 128, :])
                    lst.append(w)
            Wo_t = []
            for pair in range(2):
                wo = wp.tile([128, D], F32R, tag=f"wo{pair}", name=f"wo{pair}")
                nc.sync.dma_start(wo[:], Wo[pair * 128:(pair + 1) * 128, :])
                Wo_t.append(wo)
            bq_t, bk_t, bv_t = [], [], []
            for fb in range(2):
                for (lst, src, nm) in ((bq_t, bq, "bq"), (bk_t, bk, "bk"),
                                       (bv_t, bv, "bv")):
                    b = wp.tile([128, 1], F32, tag=f"{nm}{fb}", name=f"{nm}{fb}")
                    nc.sync.dma_start(b[:], src[fb * 128:(fb + 1) * 128, :])
                    lst.append(b)

            # ones row living at base partition 64, to pair with the
            # denominator row (psum row 64) in the replicate matmul
            ones_f = wp.tile([65, 128], F32, tag="ones_f")
            nc.gpsimd.memset(ones_f[:], 1.0)
            onesr = wp.tile([65, 128], F32R, tag="onesr")
            nc.vector.tensor_copy(onesr[:], ones_f[:])

            # ---- projection helpers (feature-major: [feat, tokens]) --------
            QT = [wp.tile([128, T], F16, tag=f"qt{fb}", name=f"qt{fb}")
                  for fb in range(2)]
            KT = [wp.tile([128, T], F16, tag=f"kt{fb}", name=f"kt{fb}")
                  for fb in range(2)]
            VT = [wp.tile([128, T], F16, tag=f"vt{fb}", name=f"vt{fb}")
                  for fb in range(2)]

            def proj_group(w_t, b_t, dst, fb, tck):
                p = ps.tile([128, 512], F32, tag="sp", name="sp")
                for dc in range(8):
                    nc.tensor.matmul(
                        p[:],
                        w_t[dc][:, fb * 128:(fb + 1) * 128],
                        xT_t[dc][:, tck * 512:(tck + 1) * 512],
                        start=(dc == 0), stop=(dc == 7))
                nc.vector.tensor_scalar_add(
                    dst[fb][:, tck * 512:(tck + 1) * 512], p[:], b_t[fb])

            # V_t[sb]: token-major [128 tokens, 4*(64+1)] with ones columns
            V_t = [wp.tile([128, VW], F16, tag=f"v{tb}", name=f"v{tb}")
                   for tb in range(NT)]

            ident = wp.tile([128, 128], F16, tag="ident")
            make_identity(nc, ident[:])

            def vt_build(sb):
                # cols h*65..h*65+63 hold V; col h*65+64 is a ones column
                # so the C psum row 64 is the softmax denominator.
                for h in range(HG):
                    nc.gpsimd.memset(V_t[sb][:, h * 65 + 64:h * 65 + 65], 1.0)
                for fb in range(2):
                    tp = ps.tile([128, 128], F16, tag="sp", name="sp")
                    nc.tensor.transpose(
                        tp[:], VT[fb][:, sb * 128:(sb + 1) * 128], ident[:])
                    for hh in range(2):
                        h = fb * 2 + hh
                        nc.vector.tensor_copy(
                            V_t[sb][:, h * 65:h * 65 + 64],
                            tp[:, hh * 64:hh * 64 + 64])

            CTn = [wp.tile([128, T], F32R, tag=f"ctn{p}", name=f"ctn{p}")
                   for p in range(2)]

            # pair-`pair` partial output projection for token block tb
            def out_unit(pair, tb):
                o = obp.tile([128, D], F32, tag="o", name="o")
                for nck in range(2):
                    p = ps.tile([128, 512], F32, tag="sp", name="sp")
                    nc.tensor.matmul(
                        p[:],
                        CTn[pair][:, tb * 128:(tb + 1) * 128],
                        Wo_t[pair][:, nck * 512:(nck + 1) * 512],
                        start=True, stop=True)
                    nc.vector.tensor_copy(o[:, nck * 512:(nck + 1) * 512], p[:])
                nc.sync.dma_start(
                    out[pair * T + tb * 128:pair * T + (tb + 1) * 128, :], o[:])

            # ---- pre-phase: Q/K fb0 and all of V^T -------------------------
            for tck in range(4):
                proj_group(Wq_t, bq_t, QT, 0, tck)
            for tck in range(4):
                proj_group(Wk_t, bk_t, KT, 0, tck)
            for fb in range(2):
                for tck in range(4):
                    proj_group(Wv_t, bv_t, VT, fb, tck)

            # ---- attention: 4 heads x 2 token-halves -----------------------
            # filler schedule per (h, half) pass: PE/DMA work from other
            # phases, interleaved to keep the PE busy while ACT runs exp.
            qk1 = ([lambda t=t: proj_group(Wq_t, bq_t, QT, 1, t)
                    for t in range(4)] +
                   [lambda t=t: proj_group(Wk_t, bk_t, KT, 1, t)
                    for t in range(4)])
            fillers = {
                (0, 0): [lambda j=j: vt_build(j) for j in range(NT)],
                (0, 1): [qk1[j // 4] if j % 4 == 0 else None for j in range(16)],
                (1, 0): [qk1[4 + j // 4] if j % 4 == 0 else None
                         for j in range(16)],
                (2, 0): [(lambda t=(j // 2): out_unit(0, t)) if j % 2 == 0
                         else None for j in range(16)],
                (2, 1): [(lambda t=(8 + j // 2): out_unit(0, t)) if j % 2 == 0
                         else None for j in range(16)],
                (3, 1): [(lambda t=(j // 2): out_unit(1, t)) if j % 2 == 0
                         else None for j in range(16)],
            }

            for h in range(HG):
                fb, ro = h // 2, (h % 2) * 64
                for half in range(2):
                    hc = half * 1024
                    ct = psc.tile([65, 1024], F32, tag="ct", name="ct")
                    pts = {}
                    fl = fillers.get((h, half), [])

                    def c_mms(j, ct=ct, h=h, pts=pts):
                        for q in range(2):
                            nc.tensor.matmul(
                                ct[:, q * 512:(q + 1) * 512],
                                V_t[j][:, h * 65:(h + 1) * 65],
                                pts[j][:, q * 512:(q + 1) * 512],
                                start=(j == 0), stop=(j == NT - 1))

                    for sb in range(NT):
                        pt = ptp.tile([128, 1024], F16, tag="pt", name="pt")
                        pts[sb] = pt
                        st = pst.tile([128, 1024], F32, tag="st", name="st")
                        for q in range(2):
                            nc.tensor.matmul(
                                st[:, q * 512:(q + 1) * 512],
                                KT[fb][ro:ro + 64, sb * 128:(sb + 1) * 128],
                                QT[fb][ro:ro + 64,
                                       hc + q * 512:hc + (q + 1) * 512],
                                start=True, stop=True)
                        nc.scalar.activation(pt[:], st[:], EXP, scale=0.125)
                        if sb < len(fl) and fl[sb] is not None:
                            fl[sb]()
                        if sb > 0:
                            c_mms(sb - 1)
                    c_mms(NT - 1)

                    # free ct fast: stage raw C + denominator to SBUF,
                    # then normalize off the critical path.
                    stg = wp.tile([65, 1024], F32R, tag=f"stg{half}",
                                  name=f"stg{half}")
                    nc.vector.tensor_copy(stg[:], ct[:])
                    for q in range(2):
                        rp = ps.tile([128, 512], F32, tag="sp", name="sp")
                        nc.tensor.matmul(
                            rp[:], onesr[64:65, :],
                            stg[64:65, q * 512:(q + 1) * 512],
                            start=True, stop=True)
                        rb = wp.tile([64, 512], F32, tag=f"rb{q}", name=f"rb{q}")
                        nc.vector.reciprocal(rb[:], rp[0:64, :])
                        nc.vector.tensor_mul(
                            CTn[fb][ro:ro + 64,
                                    hc + q * 512:hc + (q + 1) * 512],
                            stg[0:64, q * 512:(q + 1) * 512],
                            rb[:])

            # ---- remaining pair-1 output projection ------------------------
            for tb in range(8, NT):
                out_unit(1, tb)

    _split_waits(nc)
    return nc


_NC = None


def _get_nc():
    global _NC
    if _NC is None:
        _NC = _build_program()
    return _NC


def _shard_inputs(x, Wq, bq, Wk, bk, Wv, bv, Wo):
    xTs = [np.ascontiguousarray(x[b].T).astype(np.float16) for b in range(2)]
    in_maps = []
    for core in range(8):
        b, g = divmod(core, 4)
        lo = g * GF
        in_maps.append({
            "xT": xTs[b],
            "Wq": np.ascontiguousarray(Wq[:, lo:lo + GF]).astype(np.float16),
            "Wk": np.ascontiguousarray(Wk[:, lo:lo + GF]).astype(np.float16),
            "Wv": np.ascontiguousarray(Wv[:, lo:lo + GF]).astype(np.float16),
            "Wo": np.ascontiguousarray(Wo[lo:lo + GF, :]),
            "bq": np.ascontiguousarray(bq[lo:lo + GF].reshape(GF, 1)),
            "bk": np.ascontiguousarray(bk[lo:lo + GF].reshape(GF, 1)),
            "bv": np.ascontiguousarray(bv[lo:lo + GF].reshape(GF, 1)),
        })
    return in_maps


def run(inputs, trace=False, trace_kwargs=None):
    """Run the kernel; returns (output [2,2048,1024] f32, BassKernelResults)."""
    inputs = {k: np.asarray(v, dtype=np.float32) for k, v in inputs.items()}
    in_maps = _shard_inputs(
        inputs["x"], inputs["Wq"], inputs["bq"], inputs["Wk"], inputs["bk"],
        inputs["Wv"], inputs["bv"], inputs["Wo"])
    nc = _get_nc()
    res = run_bass_kernel_spmd(
        nc, in_maps, list(range(8)), trace=trace, **(trace_kwargs or {}))
    bo = inputs["bo"]
    out = np.empty((2, T, D), dtype=np.float32)
    for b in range(2):
        acc = None
        for g in range(4):
            part = res.results[4 * b + g]["out"]
            for pair in range(2):
                piece = part[pair * T:(pair + 1) * T]
                acc = piece.astype(np.float32).copy() if acc is None else acc + piece
        out[b] = acc + bo[None, :]
    return out, res


def kernel(**inputs):
    out, _ = run(inputs, trace=False)
    return out



# revision 15
# speedup vs baseline: 1.0213x; 1.0213x over previous
"""Multi-head attention (b=2, t=2048, d=1024, h=16, hd=64) on 8 trn2 NeuronCores.

Sharding: core c = 4*b + g handles batch b and head-group g (4 heads,
feature columns [g*256, (g+1)*256)).  QKV weights column-sharded, Wo
row-sharded (Megatron); each core returns two partial [2048, 1024] f16
outputs (head-pair 0 / 1 of its group) that the host sums, plus bo.

Schedule: a single 128-iteration software pipeline over (head, half,
k-block): iteration k emits S(k+1) scores -> exp(k) -> filler units ->
C(k-1) context, so the ACT engine (exp is the roofline: 128 x ~1.15us)
runs back-to-back while the PE fills score/context matmuls plus
budget-capped filler units (projections, V builds, output projections,
softmax normalizes) inside each exp window.

V is produced directly in token-major layout by projecting with a
zero-column-augmented Wv (4 zero cols) plus a broadcast bias tile that
also carries the ones columns used to accumulate softmax denominators
in the context matmul (psum row 64 per head).  Softmax skips
max-subtraction: scores are q.k/8 with q,k ~ N(0,1).  Reciprocals use
the fast custom-DVE approx (~5x cheaper than the iterative divide).
"""

import numpy as np

import concourse.bass as bass
import concourse.mybir as mybir
import concourse.tile as tile
from concourse.bass_utils import run_bass_kernel_spmd

F32 = mybir.dt.float32
F32R = mybir.dt.float32r
F16 = mybir.dt.float16
EXP = mybir.ActivationFunctionType.Exp

T = 2048          # tokens per batch
D = 1024          # model dim
HG = 4            # heads per core
HD = 64           # head dim
GF = HG * HD      # 256 features per head-group
VW = HG * (HD + 1)  # 260: V columns + a ones column per head
NT = T // 128     # 16 token blocks
NK = 128          # total pipeline iterations (8 passes x 16 k-blocks)

MAX_WAITS = 1


def _split_waits(nc):
    """walrus in this container allows only one sync-wait per instruction;
    hoist extras onto same-engine NoOps immediately before the offender."""
    for f in nc.m.functions:
        for blk in f.blocks:
            insts = list(blk.instructions)
            new, changed = [], False
            for ins in insts:
                si = ins.sync_info
                waits = list(si.on_wait) if si and si.on_wait else []
                if len(waits) > MAX_WAITS:
                    changed = True
                    extra, keep = waits[:-MAX_WAITS], waits[-MAX_WAITS:]
                    for i in range(0, len(extra), MAX_WAITS):
                        new.append(mybir.InstNoOp(
                            name=f"{ins.name}-wsplit{i}",
                            engine=ins.engine,
                            sync_info=mybir.SyncInfo(
                                on_wait=extra[i:i + MAX_WAITS], on_update=[]),
                        ))
                    ins.sync_info = mybir.SyncInfo(
                        on_wait=keep,
                        on_update=list(si.on_update) if si.on_update else [])
                new.append(ins)
            if changed:
                blk.instructions = new


def _build_program():
    nc = bass.Bass("TRN2", target_bir_lowering=False, debug=False, num_devices=8)

    xT = nc.dram_tensor("xT", [D, T], F16, kind="ExternalInput")
    Wq = nc.dram_tensor("Wq", [D, GF], F16, kind="ExternalInput")
    Wk = nc.dram_tensor("Wk", [D, GF], F16, kind="ExternalInput")
    Wv = nc.dram_tensor("Wv", [D, VW], F16, kind="ExternalInput")
    vbias = nc.dram_tensor("vbias", [128, VW], F16, kind="ExternalInput")
    Wo = nc.dram_tensor("Wo", [GF, D], F32R, kind="ExternalInput")
    bq = nc.dram_tensor("bq", [GF, 1], F32, kind="ExternalInput")
    bk = nc.dram_tensor("bk", [GF, 1], F32, kind="ExternalInput")
    # both head-pair partials: [pair*T + t, D], f16
    out = nc.dram_tensor("out", [2 * T, D], F16, kind="ExternalOutput")

    with tile.TileContext(nc) as tc:
        with (
            nc.allow_low_precision(reason="fp16/f32r rounding is intentional"),
            tc.tile_pool(name="w", bufs=1) as wp,       # persistent tiles
            tc.tile_pool(name="xt", bufs=8) as xp,      # xT tiles
            tc.tile_pool(name="pt", bufs=4) as ptp,     # probs tiles
            tc.tile_pool(name="ob", bufs=3) as obp,     # out staging
            tc.tile_pool(name="ps", bufs=2, space="PSUM") as ps,    # scratch
            tc.tile_pool(name="pst", bufs=2, space="PSUM") as pst,  # S tiles
            tc.tile_pool(name="psc", bufs=1, space="PSUM") as psc,  # C accum
        ):
            # ---- input DMAs (emission order = arrival priority) ------------
            xT_t = [xp.tile([128, T], F16, tag="xt", name=f"xt{dc}")
                    for dc in range(8)]
            Wq_t, Wk_t, Wv_t = [], [], []
            for dc in range(8):   # token-half 0 of x first
                nc.sync.dma_start(xT_t[dc][:, 0:1024], xT[dc * 128:(dc + 1) * 128, 0:1024])
            for dc in range(8):
                w = wp.tile([128, GF], F16, tag=f"wq{dc}", name=f"wq{dc}")
                nc.sync.dma_start(w[:], Wq[dc * 128:(dc + 1) * 128, :])
                Wq_t.append(w)
                w = wp.tile([128, GF], F16, tag=f"wk{dc}", name=f"wk{dc}")
                nc.sync.dma_start(w[:], Wk[dc * 128:(dc + 1) * 128, :])
                Wk_t.append(w)
            bq_t, bk_t = [], []
            for fb in range(2):
                for (lst, src, nm) in ((bq_t, bq, "bq"), (bk_t, bk, "bk")):
                    b = wp.tile([128, 1], F32, tag=f"{nm}{fb}", name=f"{nm}{fb}")
                    nc.sync.dma_start(b[:], src[fb * 128:(fb + 1) * 128, :])
                    lst.append(b)
            for dc in range(8):   # token-half 1 of x
                nc.sync.dma_start(xT_t[dc][:, 1024:2048], xT[dc * 128:(dc + 1) * 128, 1024:2048])
            for dc in range(8):
                w = wp.tile([128, VW], F16, tag=f"wv{dc}", name=f"wv{dc}")
                nc.sync.dma_start(w[:], Wv[dc * 128:(dc + 1) * 128, :])
                Wv_t.append(w)
            vb = wp.tile([128, VW], F16, tag="vb", name="vb")
            nc.sync.dma_start(vb[:], vbias[:, :])
            Wo_t = []
            for pair in range(2):
                wo = wp.tile([128, D], F32R, tag=f"wo{pair}", name=f"wo{pair}")
                nc.sync.dma_start(wo[:], Wo[pair * 128:(pair + 1) * 128, :])
                Wo_t.append(wo)

            # ones row at base partition 64 (pairs with the denominator row
            # of the C psum in the replicate matmul)
            ones_f = wp.tile([65, 128], F32, tag="ones_f", name="ones_f")
            nc.gpsimd.memset(ones_f[64:65, :], 1.0)
            onesr = wp.tile([65, 128], F32R, tag="onesr", name="onesr")
            nc.vector.tensor_copy(onesr[64:65, :], ones_f[64:65, :])

            # ---- persistent compute tiles ----------------------------------
            QT = [wp.tile([128, T], F16, tag=f"qt{fb}", name=f"qt{fb}")
                  for fb in range(2)]
            KT = [wp.tile([128, T], F16, tag=f"kt{fb}", name=f"kt{fb}")
                  for fb in range(2)]
            V_t = [wp.tile([128, VW], F16, tag=f"v{tb}", name=f"v{tb}")
                   for tb in range(NT)]
            CTn = [wp.tile([128, T], F32R, tag=f"ctn{p}", name=f"ctn{p}")
                   for p in range(2)]

            # ---- unit emitters ---------------------------------------------
            def proj_qk(w_t, b_t, dst, fb, tck, dclo, dchi, state):
                """partial feature-major projection (dc chunks [dclo,dchi))"""
                if dclo == 0:
                    state["p"] = ps.tile([128, 512], F32, tag="sp", name="sp")
                p = state["p"]
                for dc in range(dclo, dchi):
                    nc.tensor.matmul(
                        p[:],
                        w_t[dc][:, fb * 128:(fb + 1) * 128],
                        xT_t[dc][:, tck * 512:(tck + 1) * 512],
                        start=(dc == 0), stop=(dc == 7))
                if dchi == 8:
                    nc.vector.tensor_scalar_add(
                        dst[fb][:, tck * 512:(tck + 1) * 512], p[:], b_t[fb])

            def proj_v(j, dclo, dchi, state):
                """token-major V projection for token block j"""
                if dclo == 0:
                    state["p"] = ps.tile([128, VW], F32, tag="sp", name="sp")
                p = state["p"]
                for dc in range(dclo, dchi):
                    nc.tensor.matmul(
                        p[:],
                        xT_t[dc][:, j * 128:(j + 1) * 128],
                        Wv_t[dc][:],
                        start=(dc == 0), stop=(dc == 7))
                if dchi == 8:
                    nc.vector.tensor_add(V_t[j][:], p[:], vb[:])

            sts, pts, cts, rds = {}, {}, {}, {}
            stgs = {}

            def s_unit(k):
                p, sb = divmod(k, NK // 8)
                h, half = p // 2, p % 2
                fb, ro, hc = h // 2, (h % 2) * 64, half * 1024
                st = pst.tile([128, 1024], F32, tag="st", name="st")
                sts[k] = st
                for q in range(2):
                    nc.tensor.matmul(
                        st[:, q * 512:(q + 1) * 512],
                        KT[fb][ro:ro + 64, sb * 128:(sb + 1) * 128],
                        QT[fb][ro:ro + 64, hc + q * 512:hc + (q + 1) * 512],
                        start=True, stop=True)

            def exp_unit(k):
                pt = ptp.tile([128, 1024], F16, tag="pt", name="pt")
                nc.scalar.activation(pt[:], sts.pop(k)[:], EXP, scale=0.125)
                pts[k] = pt

            def c_unit(k):
                p, sb = divmod(k, NK // 8)
                h = p // 2
                if sb == 0:
                    cts[p] = psc.tile([65, 1024], F32, tag="ct", name="ct")
                ct = cts[p]
                pt = pts.pop(k)
                for q in range(2):
                    nc.tensor.matmul(
                        ct[:, q * 512:(q + 1) * 512],
                        V_t[sb][:, h * 65:(h + 1) * 65],
                        pt[:, q * 512:(q + 1) * 512],
                        start=(sb == 0), stop=(sb == NT - 1))

            def stage_ct(p):
                stg = wp.tile([65, 1024], F32R, tag=f"stg{p % 2}",
                              name=f"stg{p % 2}")
                nc.vector.tensor_copy(stg[:], cts.pop(p)[:])
                stgs[p] = stg

            def norm_unit(p, q):
                """softmax-normalize one 512-token q-chunk of pass p"""
                h, half = p // 2, p % 2
                fb, ro, hc = h // 2, (h % 2) * 64, half * 1024
                stg = stgs[p]
                rp = ps.tile([128, 512], F32, tag="sp", name="sp")
                nc.tensor.matmul(
                    rp[:], onesr[64:65, :], stg[64:65, q * 512:(q + 1) * 512],
                    start=True, stop=True)
                rb = wp.tile([64, 512], F32, tag=f"rb{q}", name=f"rb{q}")
                nc.vector.reciprocal(rb[:], rp[0:64, :])
                nc.vector.tensor_mul(
                    CTn[fb][ro:ro + 64, hc + q * 512:hc + (q + 1) * 512],
                    stg[0:64, q * 512:(q + 1) * 512],
                    rb[:])

            def out_unit(pair, tb):
                o = obp.tile([128, D], F16, tag="o", name="o")
                for nck in range(2):
                    p = ps.tile([128, 512], F32, tag="sp", name="sp")
                    nc.tensor.matmul(
                        p[:],
                        CTn[pair][:, tb * 128:(tb + 1) * 128],
                        Wo_t[pair][:, nck * 512:(nck + 1) * 512],
                        start=True, stop=True)
                    nc.vector.tensor_copy(o[:, nck * 512:(nck + 1) * 512], p[:])
                nc.sync.dma_start(
                    out[pair * T + tb * 128:pair * T + (tb + 1) * 128, :], o[:])

            # ---- filler queue ----------------------------------------------
            # unit: (cost_ns, min_k, deadline_k, emit_fn)
            queue = []

            def push(cost, min_k, deadline, fn):
                queue.append([cost, min_k, deadline, fn])

            # V blocks 2..7 (needed for C(j), emitted at iteration j+1)
            for j in range(2, 8):
                st_ = {}
                push(450, 0, j - 1, lambda j=j, s=st_: proj_v(j, 0, 4, s))
                push(580, 0, j - 1, lambda j=j, s=st_: proj_v(j, 4, 8, s))
            # K fb0 tck2,3 (scores sb>=8 of pass 0, S(8) emitted at iter 7)
            for tck in (2, 3):
                st_ = {}
                for i in range(4):
                    push(440, 0, 6, lambda t=tck, i=i, s=st_:
                         proj_qk(Wk_t, bk_t, KT, 0, t, 2 * i, 2 * i + 2, s))
            # V blocks 8..15
            for j in range(8, 16):
                st_ = {}
                push(450, 0, j - 1, lambda j=j, s=st_: proj_v(j, 0, 4, s))
                push(580, 0, j - 1, lambda j=j, s=st_: proj_v(j, 4, 8, s))
            # Q fb0 tck2,3 (pass 1 scores; S(16) emitted at iter 15)
            for tck in (2, 3):
                st_ = {}
                for i in range(4):
                    push(440, 0, 13, lambda t=tck, i=i, s=st_:
                         proj_qk(Wq_t, bq_t, QT, 0, t, 2 * i, 2 * i + 2, s))
            # Q/K fb1 all tcks (needed by pass 4 -> deadline k=62)
            for tck in range(4):
                for (w_t, b_t, dst) in ((Wq_t, bq_t, QT), (Wk_t, bk_t, KT)):
                    st_ = {}
                    for i in range(4):
                        push(440, 16, 62, lambda w=w_t, b=b_t, d=dst, t=tck,
                             i=i, s=st_: proj_qk(w, b, d, 1, t, 2 * i, 2 * i + 2, s))
            # pair-0 output projections (CTn[0] complete after pass 3 normalize)
            for tb in range(NT):
                push(470, 67, 10**9, lambda tb=tb: out_unit(0, tb))
            # pair-1 half-0 output projections (after pass 6 normalize)
            for tb in range(8):
                push(470, 115, 10**9, lambda tb=tb: out_unit(1, tb))

            # ---- preamble compute ------------------------------------------
            for tck in range(2):
                sq, sk = {}, {}
                proj_qk(Wq_t, bq_t, QT, 0, tck, 0, 8, sq)
                proj_qk(Wk_t, bk_t, KT, 0, tck, 0, 8, sk)
            for j in range(2):
                sv = {}
                proj_v(j, 0, 8, sv)

            # ---- main pipeline ---------------------------------------------
            BUDGET = 700.0
            s_unit(0)
            for k in range(NK):
                if k + 1 < NK:
                    s_unit(k + 1)
                exp_unit(k)
                # fillers: drain overdue units, then spend the slot budget
                budget = BUDGET
                while queue:
                    cost, min_k, deadline, fn = queue[0]
                    if deadline <= k:
                        queue.pop(0)
                        fn()
                        continue
                    if min_k <= k and budget > 0:
                        queue.pop(0)
                        fn()
                        budget -= cost
                        continue
                    break
                if k >= 1 and (k - 1) % 16 != 15:
                    c_unit(k - 1)
                if k % 16 == 15:
                    p = k // 16
                    c_unit(k)
                    stage_ct(p)
                    # normalize becomes the next slots' priority fillers
                    queue.insert(0, [900, 0, 10**9,
                                     lambda p=p: norm_unit(p, 1)])
                    queue.insert(0, [900, 0, 10**9,
                                     lambda p=p: norm_unit(p, 0)])

            # ---- drain any fillers the budget never reached ----------------
            while queue:
                queue.pop(0)[3]()

            # ---- tail: last pass normalize + pair-1 half-1 outputs ---------
            norm_unit(7, 0)
            for tb in range(8, 12):
                out_unit(1, tb)
            norm_unit(7, 1)
            for tb in range(12, 16):
                out_unit(1, tb)

    _split_waits(nc)
    return nc


_NC = None


def _get_nc():
    global _NC
    if _NC is None:
        _NC = _build_program()
    return _NC


def _shard_inputs(x, Wq, bq, Wk, bk, Wv, bv, Wo):
    xTs = [np.ascontiguousarray(x[b].T).astype(np.float16) for b in range(2)]
    in_maps = []
    for core in range(8):
        b, g = divmod(core, 4)
        lo = g * GF
        # augmented Wv: per head 64 V columns + one zero column; vbias carries
        # the bias plus 1.0 in the zero columns (ones columns of V)
        wv_aug = np.zeros((D, VW), dtype=np.float16)
        vb_row = np.zeros((VW,), dtype=np.float32)
        for h in range(HG):
            wv_aug[:, h * 65:h * 65 + 64] = Wv[:, lo + h * 64:lo + (h + 1) * 64]
            vb_row[h * 65:h * 65 + 64] = bv[lo + h * 64:lo + (h + 1) * 64]
            vb_row[h * 65 + 64] = 1.0
        vbias_t = np.broadcast_to(
            vb_row.astype(np.float16), (128, VW)).copy()
        in_maps.append({
            "xT": xTs[b],
            "Wq": np.ascontiguousarray(Wq[:, lo:lo + GF]).astype(np.float16),
            "Wk": np.ascontiguousarray(Wk[:, lo:lo + GF]).astype(np.float16),
            "Wv": wv_aug,
            "vbias": vbias_t,
            "Wo": np.ascontiguousarray(Wo[lo:lo + GF, :]),
            "bq": np.ascontiguousarray(bq[lo:lo + GF].reshape(GF, 1)),
            "bk": np.ascontiguousarray(bk[lo:lo + GF].reshape(GF, 1)),
        })
    return in_maps


def run(inputs, trace=False, trace_kwargs=None):
    """Run the kernel; returns (output [2,2048,1024] f32, BassKernelResults)."""
    inputs = {k: np.asarray(v, dtype=np.float32) for k, v in inputs.items()}
    in_maps = _shard_inputs(
        inputs["x"], inputs["Wq"], inputs["bq"], inputs["Wk"], inputs["bk"],
        inputs["Wv"], inputs["bv"], inputs["Wo"])
    nc = _get_nc()
    res = run_bass_kernel_spmd(
        nc, in_maps, list(range(8)), trace=trace, **(trace_kwargs or {}))
    bo = inputs["bo"]
    out = np.empty((2, T, D), dtype=np.float32)
    for b in range(2):
        acc = None
        for g in range(4):
            part = res.results[4 * b + g]["out"]
            for pair in range(2):
                piece = part[pair * T:(pair + 1) * T].astype(np.float32)
                acc = piece.copy() if acc is None else acc + piece
        out[b] = acc + bo[None, :]
    return out, res


def kernel(**inputs):
    out, _ = run(inputs, trace=False)
    return out
